# revision 7
# baseline (speedup 1.0000x reference)
"""IsoMax pairwise-distance kernel for 8 TRN2 NeuronCores.

Math:  out[b,m] = -|s| * sqrt(max(||xn_b||^2 + ||pn_m||^2 - 2*xn_b.pn_m, 0))
with xn/pn L2-normalized rows of x [4096,2048] and prototypes [12893,2048].
Since xn,pn are unit vectors this is -|s|*sqrt(2 - 2*cos).

fp8 path: G = fp8(x) @ fp8(pn*16)^T via DoubleRow perf mode (2 contraction
rows per PE cycle), accumulated f32 in PSUM. Epilogue is one ACT pass:
sqrt(svec[b]*G + 2s^2) with svec = -2s^2/(16*||x_b||). The device returns
+|s|*dist in bf16; the host negates during the f32 upcast (free).

Sharding: prototypes split across the 8 cores (output columns), x replicated.
M=12893 padded to 13312 = 8*1664 (zero rows -> harmless, sliced off on host).

Engine layout (steady-state b-loop, ~3.6-4us/tile pace):
  SWDGE(Pool): x f32->bf16 loads only (prefetches during prologue)
  Sync HWDGE:  xT/pT transposes (+6 p loads in prologue)
  ACT HWDGE:   7 p loads (prologue), p norms, 4-chunk sqrt epilogue, out store
  DVE:         x sum-sq (tensor_tensor_reduce), reciprocal, svec, bf16->fp8 casts
  PE:          32 DoubleRow matmuls (4 psum chunks x 8 k-pairs)
"""

import os
import sys

sys.path.insert(0, "/opt/trn_rl_repo")

import numpy as np

B = 4096
D = 2048
M_FULL = 12893
N_CORES = 8
MC = 1664  # per-core prototype rows (13*128); 8*1664 = 13312 >= 12893
P = 128
KT = D // P  # 16 contraction chunks
MT = MC // P  # 13 m-tiles per core
BT = B // P  # 32 b-tiles

SCALE_P = 16.0  # fp8 range scaling for normalized prototypes

_cache = {}


def _build(s_abs: float, b_rows: int = B, mc: int = MC):
    import concourse.bass as bass  # noqa: F401
    import concourse.mybir as mybir
    import concourse.tile as tile
    from concourse import bacc
    from contextlib import ExitStack

    f32 = mybir.dt.float32
    bf16 = mybir.dt.bfloat16
    fp8 = mybir.dt.float8e4
    AF = mybir.ActivationFunctionType
    PM = mybir.MatmulPerfMode
    ALU = mybir.AluOpType
    kt = D // P
    mt_n = mc // P
    bt_n = b_rows // P
    two_s2 = 2.0 * s_abs * s_abs

    # psum chunks over mc columns (<=512 wide, multiples of 128)
    chunks = []
    off = 0
    while off < mc:
        w = min(512, mc - off)
        chunks.append((off, w))
        off += w

    nc = bacc.Bacc(None, target_bir_lowering=False)
    x_d = nc.dram_tensor("x", [b_rows, D], f32, kind="ExternalInput")
    p_d = nc.dram_tensor("p", [mc, D], f32, kind="ExternalInput")
    o_d = nc.dram_tensor("o", [b_rows, mc], bf16, kind="ExternalOutput")

    with ExitStack() as ctx:
        tc = ctx.enter_context(tile.TileContext(nc))
        persist = ctx.enter_context(tc.tile_pool(name="persist", bufs=1))
        # bufs must cover every p load emitted on the ACT queue before the
        # first Square (which frees buffers) — fewer deadlocks the pipeline
        ppool = ctx.enter_context(tc.tile_pool(name="ppool", bufs=7))
        ptmp = ctx.enter_context(tc.tile_pool(name="ptmp", bufs=2))
        sq = ctx.enter_context(tc.tile_pool(name="sq", bufs=2))
        small = ctx.enter_context(tc.tile_pool(name="small", bufs=8))
        xpool = ctx.enter_context(tc.tile_pool(name="xpool", bufs=6))
        xtpool = ctx.enter_context(tc.tile_pool(name="xtpool", bufs=4))
        x8pool = ctx.enter_context(tc.tile_pool(name="x8pool", bufs=4))
        opool = ctx.enter_context(tc.tile_pool(name="opool", bufs=3))
        psum = ctx.enter_context(tc.tile_pool(name="psum", bufs=8, space="PSUM"))

        # prototypes, normalized*16, fp8, transposed: [d_inner, k, m]
        pT8 = persist.tile([P, kt, mc], fp8)

        # bias tiles for ACT (const-AP db has no arbitrary constants)
        tiny_b = persist.tile([P, 1], f32, tag="tiny_b")
        nc.vector.memset(tiny_b, 1e-30)
        two_s2_b = persist.tile([P, 1], f32, tag="two_s2_b")
        nc.vector.memset(two_s2_b, two_s2)

        # ---- prologue: load + normalize + transpose + quantize prototypes.
        # Loads split across the two HWDGE queues (ACT and Sync) so neither
        # sequencer's embedded waits serialize the reads.
        half = (mt_n + 1) // 2
        p_tiles = []
        for mt in range(mt_n):
            p_f = ppool.tile([P, D], f32, tag="p_f", name=f"p_{mt}")
            eng = nc.scalar if mt < half else nc.sync
            eng.dma_start(p_f, p_d[mt * P : (mt + 1) * P, :])
            p_tiles.append(p_f)
        for mt in range(mt_n):
            p_f = p_tiles[mt]
            psq = sq.tile([P, D], bf16, tag="sq")
            ssp = small.tile([P, 1], f32, tag="ss")
            # ssp = sum_d p^2 (Square+Sqrt share one ACT table set)
            nc.scalar.activation(psq, p_f, AF.Square, accum_out=ssp)
            pnorm = small.tile([P, 1], f32, tag="nrm")
            nc.scalar.activation(pnorm, ssp, AF.Sqrt, bias=tiny_b)
            rp = small.tile([P, 1], f32, tag="rp")
            nc.vector.reciprocal(rp, pnorm)
            rps = small.tile([P, 1], f32, tag="rps")
            nc.vector.tensor_scalar_mul(rps, rp, SCALE_P)
            pn = ptmp.tile([P, D], bf16, tag="pn")
            nc.vector.tensor_scalar_mul(pn, p_f, rps)
            pT_bf = ptmp.tile([P, kt, P], bf16, tag="pT_bf")
            nc.sync.dma_start(pT_bf, pn, transpose=True)
            # cast into the persistent fp8 slab (strided dst) on DVE
            nc.vector.tensor_scalar_mul(
                pT8[:, :, mt * P : (mt + 1) * P], pT_bf, 1.0
            )

        # ---- b-loop
        for bt in range(bt_n):
            x_bf = xpool.tile([P, D], bf16, tag="x_bf")
            # SWDGE dma casts f32 -> bf16 in flight
            nc.gpsimd.dma_start(x_bf, x_d[bt * P : (bt + 1) * P, :])
            xsq = sq.tile([P, D], bf16, tag="sq")
            ssx = small.tile([P, 1], f32, tag="ss")
            nc.scalar.activation(xsq, x_bf, AF.Square, accum_out=ssx)
            xnorm = small.tile([P, 1], f32, tag="nrm")
            nc.scalar.activation(xnorm, ssx, AF.Sqrt, bias=tiny_b)
            rx = small.tile([P, 1], f32, tag="rx")
            nc.vector.reciprocal(rx, xnorm)
            svec = small.tile([P, 1], f32, tag="svec")
            nc.vector.tensor_scalar_mul(svec, rx, -two_s2 / SCALE_P)

            xT_bf = xtpool.tile([P, kt, P], bf16, tag="xT")
            nc.sync.dma_start(xT_bf, x_bf, transpose=True)
            xT8 = x8pool.tile([P, kt, P], fp8, tag="xT8")
            nc.vector.tensor_scalar_mul(xT8, xT_bf, 1.0)

            pts = [
                psum.tile([P, 512], f32, tag="ps", name=f"ps_{ci}")[:, :w]
                for ci, (_o, w) in enumerate(chunks)
            ]
            # chunk-major: finish one PSUM bank's accumulation group first so
            # the ACT epilogue starts while later chunks still matmul
            for ci, (coff, w) in enumerate(chunks):
                for j in range(kt // 2):
                    nc.tensor.matmul(
                        pts[ci],
                        xT8[:, 2 * j : 2 * j + 2, :],
                        pT8[:, 2 * j : 2 * j + 2, coff : coff + w],
                        start=(j == 0),
                        stop=(j == kt // 2 - 1),
                        perf_mode=PM.DoubleRow,
                    )
            t_sb = opool.tile([P, mc], bf16, tag="t_sb")
            for ci, (coff, w) in enumerate(chunks):
                # sqrt(-2s^2/(16*||x||) * G + 2s^2) = s*sqrt(2 - 2*cos)
                nc.scalar.activation(
                    t_sb[:, coff : coff + w], pts[ci], AF.Sqrt,
                    bias=two_s2_b, scale=svec,
                )
            # store on the ACT HWDGE queue: its wait (epilogue done) is
            # already satisfied by queue order, and Sync stays transpose-only
            nc.scalar.dma_start(o_d[bt * P : (bt + 1) * P, :], t_sb)

    nc.compile()
    return nc


LAST_RESULT = None


def _run(nc, in_maps, core_ids):
    from concourse import bass_utils

    global LAST_RESULT
    trace = bool(int(os.environ.get("ISOMAX_TRACE", "0")))
    LAST_RESULT = bass_utils.run_bass_kernel_spmd(
        nc, in_maps, core_ids=core_ids, trace=trace
    )
    return LAST_RESULT.results


def kernel(x, prototypes, distance_scale):
    x = np.ascontiguousarray(np.asarray(x, dtype=np.float32))
    p = np.asarray(prototypes, dtype=np.float32)
    s_abs = float(abs(np.asarray(distance_scale).reshape(-1)[0].item()))
    m, d = p.shape
    assert (m, d) == (M_FULL, D) and x.shape == (B, D)

    key = ("fp8", s_abs)
    if key not in _cache:
        _cache[key] = _build(s_abs)
    nc = _cache[key]

    p_pad = np.zeros((N_CORES * MC, D), np.float32)
    p_pad[:m] = p
    in_maps = [
        {"x": x, "p": np.ascontiguousarray(p_pad[i * MC : (i + 1) * MC])}
        for i in range(N_CORES)
    ]
    results = _run(nc, in_maps, list(range(N_CORES)))
    out = np.concatenate(
        [np.asarray(results[i]["o"]) for i in range(N_CORES)], axis=1
    )
    # device emits +|s|*dist; negate during the f32 upcast
    return -(out[:, :m].astype(np.float32))


# revision 8
# speedup vs baseline: 1.0680x; 1.0680x over previous
"""IsoMax pairwise-distance kernel for 8 TRN2 NeuronCores.

Math:  out[b,m] = -|s| * sqrt(max(||xn_b||^2 + ||pn_m||^2 - 2*xn_b.pn_m, 0))
with xn/pn L2-normalized rows of x [4096,2048] and prototypes [12893,2048].
Since xn,pn are unit vectors this is -|s|*sqrt(2 - 2*cos).

fp8 path: G = fp8(x) @ fp8(pn*16)^T via DoubleRow perf mode (2 contraction
rows per PE cycle), accumulated f32 in PSUM. Epilogue is one ACT pass:
sqrt(svec[b]*G + 2s^2) with svec = -2s^2/(16*||x_b||). The device returns
+|s|*dist in bf16; the host negates during the f32 upcast (free).

Sharding: prototypes split across the 8 cores (output columns), x replicated.
M=12893 padded to 13312 = 8*1664 (zero rows -> harmless, sliced off on host).

The b-loop is software-pipelined to break the cross-engine serial chain
(load -> transpose -> cast -> matmul -> epilogue -> store):
  - x loads prefetch XPIPE tiles ahead on the SWDGE queue
  - transposes run TPIPE tiles ahead on Sync, ahead of the store waits
  - ACT norm work (Square/Sqrt) runs one tile ahead of the epilogue
  - the DVE fp8 cast is emitted before the norm scalars so matmuls never
    wait on the norm chain
"""

import os
import sys

sys.path.insert(0, "/opt/trn_rl_repo")

import numpy as np

B = 4096
D = 2048
M_FULL = 12893
N_CORES = 8
MC = 1664  # per-core prototype rows (13*128); 8*1664 = 13312 >= 12893
P = 128
KT = D // P  # 16 contraction chunks
MT = MC // P  # 13 m-tiles per core
BT = B // P  # 32 b-tiles

SCALE_P = 16.0  # fp8 range scaling for normalized prototypes

_cache = {}


def _build(s_abs: float, b_rows: int = B, mc: int = MC):
    import concourse.bass as bass  # noqa: F401
    import concourse.mybir as mybir
    import concourse.tile as tile
    from concourse import bacc
    from contextlib import ExitStack

    f32 = mybir.dt.float32
    bf16 = mybir.dt.bfloat16
    fp8 = mybir.dt.float8e4
    AF = mybir.ActivationFunctionType
    PM = mybir.MatmulPerfMode
    kt = D // P
    mt_n = mc // P
    bt_n = b_rows // P
    two_s2 = 2.0 * s_abs * s_abs

    # psum chunks over mc columns (<=512 wide, multiples of 128)
    chunks = []
    off = 0
    while off < mc:
        w = min(512, mc - off)
        chunks.append((off, w))
        off += w

    XPIPE = 4  # x-load prefetch depth (bounded by xpool bufs)
    TPIPE = 2  # transpose lookahead (bounded by xtpool bufs)

    nc = bacc.Bacc(None, target_bir_lowering=False)
    x_d = nc.dram_tensor("x", [b_rows, D], f32, kind="ExternalInput")
    p_d = nc.dram_tensor("p", [mc, D], f32, kind="ExternalInput")
    o_d = nc.dram_tensor("o", [b_rows, mc], bf16, kind="ExternalOutput")

    with ExitStack() as ctx:
        tc = ctx.enter_context(tile.TileContext(nc))
        persist = ctx.enter_context(tc.tile_pool(name="persist", bufs=1))
        # p loads are all emitted upfront on Sync; bufs must cover the loads
        # in flight before the first Square frees one, else the queue deadlocks
        ppool = ctx.enter_context(tc.tile_pool(name="ppool", bufs=7))
        ptmp = ctx.enter_context(tc.tile_pool(name="ptmp", bufs=2))
        sq = ctx.enter_context(tc.tile_pool(name="sq", bufs=2))
        small = ctx.enter_context(tc.tile_pool(name="small", bufs=8))
        xpool = ctx.enter_context(tc.tile_pool(name="xpool", bufs=6))
        xtpool = ctx.enter_context(tc.tile_pool(name="xtpool", bufs=4))
        x8pool = ctx.enter_context(tc.tile_pool(name="x8pool", bufs=4))
        opool = ctx.enter_context(tc.tile_pool(name="opool", bufs=3))
        psum = ctx.enter_context(tc.tile_pool(name="psum", bufs=8, space="PSUM"))

        # prototypes, normalized*16, fp8, transposed: [d_inner, k, m]
        pT8 = persist.tile([P, kt, mc], fp8)

        # bias tiles for ACT (const-AP db has no arbitrary constants)
        tiny_b = persist.tile([P, 1], f32, tag="tiny_b")
        nc.vector.memset(tiny_b, 1e-30)
        two_s2_b = persist.tile([P, 1], f32, tag="two_s2_b")
        nc.vector.memset(two_s2_b, two_s2)

        # ---- x-load prefetch + prologue p loads (wave A fills ppool) ----
        x_bfs = {}

        def load_x(i):
            if i >= bt_n:
                return
            t = xpool.tile([P, D], bf16, tag="x_bf")
            # SWDGE dma casts f32 -> bf16 in flight
            nc.gpsimd.dma_start(t, x_d[i * P : (i + 1) * P, :])
            x_bfs[i] = t

        xT_bfs = {}

        def trans_x(i):
            if i >= bt_n:
                return
            t = xtpool.tile([P, kt, P], bf16, tag="xT")
            nc.sync.dma_start(t, x_bfs[i], transpose=True)
            xT_bfs[i] = t

        for i in range(min(XPIPE, bt_n)):
            load_x(i)

        wave_a = min(7, mt_n)
        p_tiles = {}
        for mt in range(wave_a):
            p_f = ppool.tile([P, D], f32, tag="p_f")
            nc.sync.dma_start(p_f, p_d[mt * P : (mt + 1) * P, :])
            p_tiles[mt] = p_f

        # ---- prologue: normalize + transpose + quantize prototypes ----
        for mt in range(mt_n):
            p_f = p_tiles[mt]
            psq = sq.tile([P, D], bf16, tag="sq")
            ssp = small.tile([P, 1], f32, tag="ss")
            # ssp = sum_d p^2 (Square+Sqrt share one ACT table set)
            nc.scalar.activation(psq, p_f, AF.Square, accum_out=ssp)
            pnorm = small.tile([P, 1], f32, tag="nrm")
            nc.scalar.activation(pnorm, ssp, AF.Sqrt, bias=tiny_b)
            rp = small.tile([P, 1], f32, tag="rp")
            nc.vector.reciprocal(rp, pnorm)
            rps = small.tile([P, 1], f32, tag="rps")
            nc.vector.tensor_scalar_mul(rps, rp, SCALE_P)
            pn = ptmp.tile([P, D], bf16, tag="pn")
            nc.vector.tensor_scalar_mul(pn, p_f, rps)
            pT_bf = ptmp.tile([P, kt, P], bf16, tag="pT_bf")
            nc.sync.dma_start(pT_bf, pn, transpose=True)
            # cast into the persistent fp8 slab (strided dst) on DVE
            nc.vector.tensor_scalar_mul(
                pT8[:, :, mt * P : (mt + 1) * P], pT_bf, 1.0
            )
            # wave B p load now that a ppool buf is free
            nmt = mt + wave_a
            if nmt < mt_n:
                p_f2 = ppool.tile([P, D], f32, tag="p_f")
                nc.sync.dma_start(p_f2, p_d[nmt * P : (nmt + 1) * P, :])
                p_tiles[nmt] = p_f2

        for i in range(min(TPIPE, bt_n)):
            trans_x(i)

        # ---- software-pipelined b-loop ----
        # per tile i the body emits: load(i+XPIPE), trans(i+TPIPE),
        # cast(i), norms(i+1), matmuls(i), epilogue(i), store(i)
        # ACT norm work for tile 0 is emitted here (peeled iteration)
        def norms(i):
            if i >= bt_n:
                return
            xsq = sq.tile([P, D], bf16, tag="sq")
            ssx = small.tile([P, 1], f32, tag="ss")
            nc.scalar.activation(xsq, x_bfs[i], AF.Square, accum_out=ssx)
            xnorm = small.tile([P, 1], f32, tag="nrm")
            nc.scalar.activation(xnorm, ssx, AF.Sqrt, bias=tiny_b)
            rx = small.tile([P, 1], f32, tag="rx")
            nc.vector.reciprocal(rx, xnorm)
            svec = small.tile([P, 1], f32, tag="svec")
            nc.vector.tensor_scalar_mul(svec, rx, -two_s2 / SCALE_P)
            return svec

        svecs = {0: norms(0)}
        for bt in range(bt_n):
            load_x(bt + XPIPE)
            trans_x(bt + TPIPE)

            # fp8 cast first on DVE: matmuls depend only on the transpose
            xT8 = x8pool.tile([P, kt, P], fp8, tag="xT8")
            nc.vector.tensor_scalar_mul(xT8, xT_bfs.pop(bt), 1.0)

            svecs[bt + 1] = norms(bt + 1)
            x_bfs.pop(bt)

            pts = [
                psum.tile([P, 512], f32, tag="ps", name=f"ps_{ci}")[:, :w]
                for ci, (_o, w) in enumerate(chunks)
            ]
            # chunk-major: finish one PSUM bank's accumulation group first so
            # the ACT epilogue starts while later chunks still matmul
            for ci, (coff, w) in enumerate(chunks):
                for j in range(kt // 2):
                    nc.tensor.matmul(
                        pts[ci],
                        xT8[:, 2 * j : 2 * j + 2, :],
                        pT8[:, 2 * j : 2 * j + 2, coff : coff + w],
                        start=(j == 0),
                        stop=(j == kt // 2 - 1),
                        perf_mode=PM.DoubleRow,
                    )
            t_sb = opool.tile([P, mc], bf16, tag="t_sb")
            svec = svecs.pop(bt)
            for ci, (coff, w) in enumerate(chunks):
                # sqrt(-2s^2/(16*||x||) * G + 2s^2) = s*sqrt(2 - 2*cos)
                nc.scalar.activation(
                    t_sb[:, coff : coff + w], pts[ci], AF.Sqrt,
                    bias=two_s2_b, scale=svec,
                )
            nc.sync.dma_start(o_d[bt * P : (bt + 1) * P, :], t_sb)

    nc.compile()
    return nc


LAST_RESULT = None


def _run(nc, in_maps, core_ids):
    from concourse import bass_utils

    global LAST_RESULT
    trace = bool(int(os.environ.get("ISOMAX_TRACE", "0")))
    LAST_RESULT = bass_utils.run_bass_kernel_spmd(
        nc, in_maps, core_ids=core_ids, trace=trace
    )
    return LAST_RESULT.results


def kernel(x, prototypes, distance_scale):
    x = np.ascontiguousarray(np.asarray(x, dtype=np.float32))
    p = np.asarray(prototypes, dtype=np.float32)
    s_abs = float(abs(np.asarray(distance_scale).reshape(-1)[0].item()))
    m, d = p.shape
    assert (m, d) == (M_FULL, D) and x.shape == (B, D)

    key = ("fp8", s_abs)
    if key not in _cache:
        _cache[key] = _build(s_abs)
    nc = _cache[key]

    p_pad = np.zeros((N_CORES * MC, D), np.float32)
    p_pad[:m] = p
    in_maps = [
        {"x": x, "p": np.ascontiguousarray(p_pad[i * MC : (i + 1) * MC])}
        for i in range(N_CORES)
    ]
    results = _run(nc, in_maps, list(range(N_CORES)))
    out = np.concatenate(
        [np.asarray(results[i]["o"]) for i in range(N_CORES)], axis=1
    )
    # device emits +|s|*dist; negate during the f32 upcast
    return -(out[:, :m].astype(np.float32))
